# revision 4
# baseline (speedup 1.0000x reference)
"""Trainium2 Bass kernel for GPT2-style single attention layer.

Problem: B=4, S=2048, E=1024, H=16 heads, D=64.
  x = hidden @ W_attn + b_attn ; q,k,v = split(x)
  per head: softmax(causal(q k^T / 8) + mask) @ v
  out = merge @ W_proj + b_proj

Sharding over 8 cores: core i -> batch b = i//2, heads h0 = (i%2)*8 .. +8
(data parallel on B, tensor parallel over heads).  Each core's work is fully
local; the host sums the two partial projections per batch.

Dataflow is fully "transposed" so no on-chip transposes are ever needed:
  host feeds xT = hidden[b].T                       [E, S]
  Q^T,K^T = (Wq|Wk block)^T @ xT     -> [d, tok] per head   (W stationary)
  V       = xT_block^T @ Wv          -> [tok, d] natural    (xT stationary)
  S^T     = K^T_blk^T @ Q^T          -> [k, q]   (softmax dim on partitions)
  P^T     = exp(0.125*S^T + mask[k]) * causal01
  sums    = ones^T @ P^T             -> [1, q]  (ones col in V, PSUM-accum)
  attn^T  = V_blk^T @ P^T            -> [d, q]  accumulated over k tiles
  norm    = attn^T * broadcast(1/sums)   (broadcast via K=1 ones-matmul)
  out^T   = Wp_blk^T @ attn^T        -> [col, tok]
Host transposes out^T back and sums core pairs + b_proj.

Schedule: the PE (matmul) engine is the global bottleneck (~200us of work
vs ~160us ACT exp).  Attention k-loops are exp-paced, so PE idle slots are
filled from a two-tier filler queue: prep (next quarter's V/Q/K) drains
first; deferred projections are RESERVED for the last quarter, whose
attention stretch has the largest ACT-vs-PE deficit.  Startup DMAs are
ordered so the first matmul group gates on ~1.5MB, not the full 9MB.
"""

import os
import ml_dtypes
import numpy as np

B, S, E, H, D = 4, 2048, 1024, 16, 64
NC = 8
HL = H // 2          # local heads per core
EL = HL * D          # local embedding slice = 512
P = 128              # partitions
QT = 512             # q tile width (f32 moving max)
NQT = S // QT        # 4 q tiles
NKT = S // P         # 16 k tiles
NET = E // P         # 8 e (contraction) tiles

_CACHE = {}
LAST_RESULT = None


def _build(has_bv: bool):
    from contextlib import ExitStack

    import concourse.tile as tile
    from concourse import bacc, mybir

    f32 = mybir.dt.float32
    f32r = mybir.dt.bfloat16  # matmul operand dtype (2-byte: full-rate moving operand)
    EXP = mybir.ActivationFunctionType.Exp

    nc = bacc.Bacc(
        "TRN2",
        target_bir_lowering=False,
        debug=False,
        enable_asserts=False,
        num_devices=NC,
    )

    def inp(name, shape, dt=f32):
        return nc.dram_tensor(name, shape, dt, kind="ExternalInput").ap()

    xt_d = inp("xt", [E, S], f32r)
    wq_d = inp("wq", [E, EL], f32r)
    wk_d = inp("wk", [E, EL], f32r)
    wv_d = inp("wv", [E, EL], f32r)
    wp_d = inp("wp", [EL, E], f32r)
    bq_d = inp("bq", [P, 4])
    bk_d = inp("bk", [P, 4])
    bv_d = inp("bv", [P, 4])
    maskt_d = inp("maskt", [P, NKT])
    causal_d = inp("causal", [P, P], f32r)
    ones_d = inp("ones", [P, 64], f32r)
    out_d = nc.dram_tensor("out", [E, S], f32r, kind="ExternalOutput").ap()

    with tile.TileContext(nc) as tc, ExitStack() as ctx:
        const = ctx.enter_context(tc.tile_pool(name="const", bufs=1))
        big = ctx.enter_context(tc.tile_pool(name="big", bufs=1))
        wpool = ctx.enter_context(tc.tile_pool(name="wpool", bufs=1))
        xpool = ctx.enter_context(tc.tile_pool(name="xpool", bufs=1))
        ptpool = ctx.enter_context(tc.tile_pool(name="ptpool", bufs=1))
        aopool = ctx.enter_context(tc.tile_pool(name="aopool", bufs=1))
        ospool = ctx.enter_context(tc.tile_pool(name="ospool", bufs=1))
        rcpool = ctx.enter_context(tc.tile_pool(name="rcpool", bufs=1))
        aospool = ctx.enter_context(tc.tile_pool(name="aospool", bufs=1))
        psum = ctx.enter_context(tc.tile_pool(name="psum", bufs=1, space="PSUM"))

        # ---- persistent big buffers ----
        # Q^T / K^T: per head-pair p a [128, S] tile (partitions = 2 heads x 64 d)
        qt_tiles = [big.tile([P, S], f32r, name=f"qt{p}", tag=f"qt{p}") for p in range(4)]
        kt_tiles = [big.tile([P, S], f32r, name=f"kt{p}", tag=f"kt{p}") for p in range(4)]
        # V natural: 16 tiles [128 tok, 512 vcol (+ones col per head)]
        v_tiles = [big.tile([P, 8 * 65], f32r, name=f"v{t}", tag=f"v{t}") for t in range(NKT)]

        x_tiles = [[None] * NQT for _ in range(NET)]

        # ---- DMA emission: ordered by first use so early compute gates on
        # as few bytes as possible (the DMA queue drains in order).
        bq_t = const.tile([P, 4], f32, name="bq_t")
        nc.sync.dma_start(bq_t[:], bq_d[:])
        bk_t = const.tile([P, 4], f32, name="bk_t")
        nc.sync.dma_start(bk_t[:], bk_d[:])
        maskt_t = const.tile([P, NKT], f32, name="maskt_t")
        nc.sync.dma_start(maskt_t[:], maskt_d[:])
        causal_t = const.tile([P, P], f32r, name="causal_t")
        nc.sync.dma_start(causal_t[:], causal_d[:])
        ones_t = const.tile([P, 64], f32r, name="ones_t")
        nc.sync.dma_start(ones_t[:], ones_d[:])
        bv_t = const.tile([P, 4], f32, name="bv_t")
        nc.sync.dma_start(bv_t[:], bv_d[:])

        def load_x_quarter(tq):
            xb = xpool.tile([P, NET * QT], f32r, name=f"xb{tq}", tag=f"xb{tq}", bufs=1)
            nc.sync.dma_start(
                xb[:].rearrange("p (a c) -> p a c", a=NET, c=QT),
                xt_d.rearrange("(a p) s -> p a s", p=P)[:, :, tq * QT:(tq + 1) * QT],
            )
            for kt in range(NET):
                x_tiles[kt][tq] = xb[:, kt * QT:(kt + 1) * QT]

        # x quarter 0 first: gates the very first Q/K matmul groups.
        load_x_quarter(0)

        # wq/wk big tiles; ct0 column slices land first so q_ct(0,0)/k_ct(0,0)
        # can start after ~1.5MB of DMA instead of ~3MB.
        wq_b = wpool.tile([P, NET * EL], f32r, name="wb_q", tag="wb_q", bufs=1)
        wk_b = wpool.tile([P, NET * EL], f32r, name="wb_k", tag="wb_k", bufs=1)

        def load_w_slice(wb, dram, c0, c1):
            nc.sync.dma_start(
                wb[:].rearrange("p (a c) -> p a c", a=NET, c=EL)[:, :, c0:c1],
                dram.rearrange("(a p) c -> p a c", p=P)[:, :, c0:c1],
            )

        load_w_slice(wq_b, wq_d, 0, P)
        load_w_slice(wk_b, wk_d, 0, P)
        wq_t = [wq_b[:, kt * EL:(kt + 1) * EL] for kt in range(NET)]
        wk_t = [wk_b[:, kt * EL:(kt + 1) * EL] for kt in range(NET)]

        # wv full (gates the V groups of quarter 0)
        wv_b = wpool.tile([P, NET * EL], f32r, name="wb_v", tag="wb_v", bufs=1)
        nc.sync.dma_start(
            wv_b[:].rearrange("p (a c) -> p a c", a=NET, c=EL),
            wv_d.rearrange("(a p) c -> p a c", p=P),
        )
        wv_t = [wv_b[:, kt * EL:(kt + 1) * EL] for kt in range(NET)]

        # rest of wq/wk (needed by pairs 1..3 of quarter 0)
        load_w_slice(wq_b, wq_d, P, EL)
        load_w_slice(wk_b, wk_d, P, EL)

        load_x_quarter(1)

        wpb = wpool.tile([P, 4 * E], f32r, name="wpb", tag="wpb", bufs=1)
        nc.sync.dma_start(
            wpb[:].rearrange("p (a c) -> p a c", a=4, c=E),
            wp_d.rearrange("(a p) c -> p a c", p=P),
        )
        wp_tiles = [wpb[:, p * E:(p + 1) * E] for p in range(4)]

        load_x_quarter(2)
        load_x_quarter(3)

        # ---- per-group compute units (run directly or as fillers) ----
        done = set()

        def v_tt(tq, tt):
            key = ("v", tq, tt)
            if key in done:
                return
            done.add(key)
            ps = psum.tile([P, EL], f32, name=f"psv{tq}_{tt}", tag="mm", bufs=2)
            for kt in range(NET):
                nc.tensor.matmul(
                    ps[:], x_tiles[kt][tq][:, tt * P:(tt + 1) * P], wv_t[kt][:],
                    start=(kt == 0), stop=(kt == NET - 1))
            vt = v_tiles[tq * 4 + tt]
            v8 = vt[:, 0:520].rearrange("p (a c) -> p a c", a=8, c=65)
            nc.vector.tensor_copy(
                v8[:, :, 0:64], ps[:].rearrange("p (a c) -> p a c", a=8, c=64))
            nc.gpsimd.memset(v8[:, :, 64:65], 1.0)

        def q_ct(tq, ct):
            key = ("q", tq, ct)
            if key in done:
                return
            done.add(key)
            ps = psum.tile([P, QT], f32, name=f"psq{tq}_{ct}", tag="mm", bufs=2)
            for kt in range(NET):
                nc.tensor.matmul(ps[:], wq_t[kt][:, ct * P:(ct + 1) * P],
                                 x_tiles[kt][tq][:],
                                 start=(kt == 0), stop=(kt == NET - 1))
            nc.vector.tensor_scalar_add(
                qt_tiles[ct][:, tq * QT:(tq + 1) * QT], ps[:], bq_t[:, ct:ct + 1])

        def k_ct(tq, ct):
            key = ("k", tq, ct)
            if key in done:
                return
            done.add(key)
            ps = psum.tile([P, QT], f32, name=f"psk{tq}_{ct}", tag="mm", bufs=2)
            for kt in range(NET):
                nc.tensor.matmul(ps[:], wk_t[kt][:, ct * P:(ct + 1) * P],
                                 x_tiles[kt][tq][:],
                                 start=(kt == 0), stop=(kt == NET - 1))
            nc.vector.tensor_scalar_add(
                kt_tiles[ct][:, tq * QT:(tq + 1) * QT], ps[:], bk_t[:, ct:ct + 1])

        ao_map = {}

        def proj_ct(qt, ct):
            key = ("p", qt, ct)
            if key in done:
                return
            done.add(key)
            ps = psum.tile([P, QT], f32, name=f"psp{qt}_{ct}", tag="mm", bufs=2)
            for p in range(4):
                nc.tensor.matmul(ps[:], wp_tiles[p][:, ct * P:(ct + 1) * P],
                                 ao_map[(p, qt)][:], start=(p == 0), stop=(p == 3))
            osb = ospool.tile([P, QT], f32r, name=f"os{qt}_{ct}", tag="os", bufs=2)
            nc.vector.tensor_copy(osb[:], ps[:])
            nc.sync.dma_start(out_d[ct * P:(ct + 1) * P, qt * QT:(qt + 1) * QT],
                              osb[:])

        # ---- two-tier filler queue ----
        prep_fillers = []   # V/Q/K groups: drain first, anywhere
        late_fillers = []   # deferred proj groups: reserved for last quarter

        def mkfiller(fn, *args):
            def run():
                before = len(done)
                fn(*args)
                return len(done) != before
            return run

        def drain_one(allow_late):
            while prep_fillers:
                fn = prep_fillers.pop(0)
                if fn():
                    return
            if allow_late:
                while late_fillers:
                    fn = late_fillers.pop(0)
                    if fn():
                        return

        def attention(p, qt, allow_late, drain_mod):
            """Head pair p (heads 2p, 2p+1), q tile qt.

            Returns (aos, rca, rcb): unnormalized attn output (bf16 SBUF) and
            the f32 reciprocal softmax denominators for both heads."""
            kt_max = 4 * (qt + 1)
            # row 64 of each av accumulates the softmax denominator (ones col)
            ava = psum.tile([65, QT], f32, name=f"ava{p}_{qt}", tag="ava", bufs=1)
            avb = psum.tile([65, QT], f32, name=f"avb{p}_{qt}", tag="avb", bufs=1)

            def av_sums(kt, pt, off):
                first, last = kt == 0, kt == kt_max - 1
                vva = v_tiles[kt][:, (2 * p) * 65:(2 * p + 1) * 65]
                vvb = v_tiles[kt][:, (2 * p + 1) * 65:(2 * p + 2) * 65]
                nc.tensor.matmul(ava[:, off:QT], vva, pt[:, off:QT],
                                 start=first, stop=last)
                nc.tensor.matmul(avb[:, off:QT], vvb, pt[:, QT + off:2 * QT],
                                 start=first, stop=last)

            pending = None
            for kt in range(kt_max):
                # diagonal tiles: only q columns >= off are unmasked
                diag = kt >= qt * 4
                off = (kt - qt * 4) * P if diag else 0
                kl = slice(kt * P, (kt + 1) * P)
                qv = slice(qt * QT + off, (qt + 1) * QT)
                st = psum.tile([P, 2 * QT], f32, name=f"st{p}_{qt}_{kt}",
                               tag="st", bufs=2)
                nc.tensor.matmul(st[:, off:QT], kt_tiles[p][0:64, kl],
                                 qt_tiles[p][0:64, qv])
                nc.tensor.matmul(st[:, QT + off:2 * QT], kt_tiles[p][64:128, kl],
                                 qt_tiles[p][64:128, qv])
                pt = ptpool.tile([P, 2 * QT], f32r, name=f"pt{p}_{qt}_{kt}",
                                 tag="pt", bufs=5)
                bias = maskt_t[:, kt:kt + 1]
                if not diag or off == 0:
                    nc.scalar.activation(pt[:], st[:], EXP, bias=bias, scale=0.125)
                else:
                    stv = st[:].rearrange("p (h q) -> p h q", h=2, q=QT)[:, :, off:QT]
                    ptv = pt[:].rearrange("p (h q) -> p h q", h=2, q=QT)[:, :, off:QT]
                    nc.scalar.activation(ptv, stv, EXP, bias=bias, scale=0.125)
                if diag:
                    # triangular band at the leading 128 valid columns
                    nc.vector.tensor_mul(pt[:, off:off + P], pt[:, off:off + P],
                                         causal_t[:])
                    nc.vector.tensor_mul(pt[:, QT + off:QT + off + P],
                                         pt[:, QT + off:QT + off + P], causal_t[:])
                if pending is not None:
                    av_sums(*pending)
                    if kt % drain_mod == 0:
                        drain_one(allow_late)
                pending = (kt, pt, off)
            av_sums(*pending)

            # drain PSUM immediately so the next pair's AV can start:
            # attn halves -> bf16 SBUF, denominators -> SBUF, then 1/sums
            # (DVE) and bf16 copies (GpSimd) so the later broadcast matmuls
            # gate on nothing slow.
            aos = aospool.tile([P, QT], f32r, name=f"aos{p}_{qt}",
                               tag=f"aos{p}", bufs=2)
            nc.vector.tensor_copy(aos[0:64, :], ava[0:64, :])
            nc.vector.tensor_copy(aos[64:128, :], avb[0:64, :])
            sga = rcpool.tile([1, QT], f32, name=f"sga{p}_{qt}", tag="sga", bufs=2)
            sgb = rcpool.tile([1, QT], f32, name=f"sgb{p}_{qt}", tag="sgb", bufs=2)
            nc.vector.tensor_copy(sga[:], ava[64:65, :])
            nc.vector.tensor_copy(sgb[:], avb[64:65, :])
            rcf = rcpool.tile([1, QT], f32, name=f"rcf{p}_{qt}", tag="rcf", bufs=2)
            rcg = rcpool.tile([1, QT], f32, name=f"rcg{p}_{qt}", tag="rcg", bufs=2)
            nc.vector.reciprocal_approx_fast(rcf[:], sga[:])
            nc.vector.reciprocal_approx_fast(rcg[:], sgb[:])
            rba = rcpool.tile([1, QT], f32r, name=f"rba{p}_{qt}", tag="rba", bufs=2)
            rbb = rcpool.tile([1, QT], f32r, name=f"rbb{p}_{qt}", tag="rbb", bufs=2)
            nc.gpsimd.tensor_copy(rba[:], rcf[:])
            nc.gpsimd.tensor_copy(rbb[:], rcg[:])
            return aos, rba, rbb

        def finish_norm(p, qt, aos, rba, rbb):
            """Broadcast 1/sums over 64 partitions (K=1 ones-matmuls) and
            scale the attention output.  Runs as a filler; its inputs were
            prepared at the pair's attention end so the matmuls don't stall
            the in-order PE queue."""
            key = ("n", p, qt)
            if key in done:
                return
            done.add(key)
            ao = aopool.tile([P, QT], f32r, name=f"ao{p}_{qt}", tag=f"ao{p}",
                             bufs=4)
            for half, rcx in ((0, rba), (1, rbb)):
                rb = psum.tile([64, QT], f32, name=f"rb{p}_{qt}_{half}",
                               tag="mm", bufs=2)
                nc.tensor.matmul(rb[:], ones_t[0:1, 0:64], rcx[:],
                                 tile_position=(0, 0))
                nc.vector.tensor_mul(ao[64 * half:64 * (half + 1), :], rb[:],
                                     aos[64 * half:64 * (half + 1), :])
            if has_bv:
                nc.vector.tensor_scalar_add(ao[:], ao[:], bv_t[:, p:p + 1])
            ao_map[(p, qt)] = ao

        # ============ main schedule ============
        pending_norm = None
        for tq in range(NQT):
            for tt in range(4):
                v_tt(tq, tt)
            q_ct(tq, 0)
            k_ct(tq, 0)
            for ct in range(1, 4):
                prep_fillers.append(mkfiller(q_ct, tq, ct))
                prep_fillers.append(mkfiller(k_ct, tq, ct))
            if tq + 1 < NQT:
                for tt in range(4):
                    prep_fillers.append(mkfiller(v_tt, tq + 1, tt))
                for ct in range(4):
                    prep_fillers.append(mkfiller(q_ct, tq + 1, ct))
                    prep_fillers.append(mkfiller(k_ct, tq + 1, ct))
            allow_late = tq == NQT - 1
            drain_mod = 2 if tq == 0 else 1  # early DMAs still landing in tq0
            for p in range(4):
                q_ct(tq, p)
                k_ct(tq, p)
                if pending_norm is not None:
                    # drained early in the upcoming attention; its DVE inputs
                    # (the reciprocals) are ready by the first drain slot
                    prep_fillers.insert(min(2, len(prep_fillers)),
                                        mkfiller(finish_norm, *pending_norm))
                    pending_norm = None
                pending_norm = (p, tq, *attention(p, tq, allow_late, drain_mod))
            if tq < NQT - 1:
                # quarter's proj groups: reserved for the last quarter's
                # ACT-bound attention stretch.  drain_one always exhausts
                # prep (which holds every finish_norm of this quarter) before
                # touching these, so ao_map is complete when they run.
                for ct in range(NET):
                    late_fillers.append(mkfiller(proj_ct, tq, ct))

        # ---- end of quarters: finish pending norm and flush remaining work
        # By construction pending_norm is pair (3, NQT-1).
        finish_norm(*pending_norm)
        for fn in list(prep_fillers):
            fn()
        prep_fillers.clear()
        for fn in list(late_fillers):
            fn()
        late_fillers.clear()
        for qt in range(NQT):
            for ct in range(NET):
                proj_ct(qt, ct)

    nc.compile()
    return nc


def _causal_tiles():
    """[128, 128] lower-triangular 0/1 band mask (dq >= dk)."""
    dk = np.arange(P)[:, None]
    dq = np.arange(P)[None, :]
    return np.ascontiguousarray((dq >= dk).astype(np.float32))


def kernel(hidden_state, attention_mask, W_attn, b_attn, W_proj, b_proj):
    global LAST_RESULT
    hs = np.asarray(hidden_state, np.float32)
    am = np.asarray(attention_mask, np.float32).reshape(B, S)
    wa = np.asarray(W_attn, np.float32)
    ba = np.asarray(b_attn, np.float32)
    wpr = np.asarray(W_proj, np.float32)
    bp = np.asarray(b_proj, np.float32)

    has_bv = bool(np.any(ba[2 * E:3 * E] != 0.0))
    key = ("k", has_bv)
    if key not in _CACHE:
        _CACHE[key] = _build(has_bv)
    nc = _CACHE[key]

    bf16 = ml_dtypes.bfloat16
    causal = _causal_tiles().astype(bf16)
    in_maps = []
    for core in range(NC):
        b = core // 2
        c0 = (core % 2) * EL
        in_maps.append({
            "xt": np.ascontiguousarray(hs[b].T).astype(bf16),
            "wq": np.ascontiguousarray(wa[:, c0:c0 + EL]).astype(bf16),
            "wk": np.ascontiguousarray(wa[:, E + c0:E + c0 + EL]).astype(bf16),
            "wv": np.ascontiguousarray(wa[:, 2 * E + c0:2 * E + c0 + EL]).astype(bf16),
            "wp": np.ascontiguousarray(wpr[c0:c0 + EL, :]).astype(bf16),
            "bq": np.ascontiguousarray(ba[c0:c0 + EL].reshape(4, P).T),
            "bk": np.ascontiguousarray(ba[E + c0:E + c0 + EL].reshape(4, P).T),
            "bv": np.ascontiguousarray(ba[2 * E + c0:2 * E + c0 + EL].reshape(4, P).T),
            "maskt": np.ascontiguousarray(am[b].reshape(NKT, P).T),
            "causal": causal,
            "ones": np.ones((P, 64), bf16),
        })

    from concourse.bass_utils import run_bass_kernel_spmd

    trace = os.environ.get("KERNEL_TRACE", "") == "1"
    res = run_bass_kernel_spmd(nc, in_maps, core_ids=list(range(NC)), trace=trace)
    LAST_RESULT = res

    full = np.empty((B, S, E), np.float32)
    for b in range(B):
        full[b] = res.results[2 * b]["out"].T.astype(np.float32)
        full[b] += res.results[2 * b + 1]["out"].T.astype(np.float32)
        full[b] += bp
    return full


# revision 5
# speedup vs baseline: 1.1517x; 1.1517x over previous
"""Trainium2 Bass kernel for GPT2-style single attention layer.

Problem: B=4, S=2048, E=1024, H=16 heads, D=64.
  x = hidden @ W_attn + b_attn ; q,k,v = split(x)
  per head: softmax(causal(q k^T / 8) + mask) @ v
  out = merge @ W_proj + b_proj

Sharding over 8 cores: core i -> batch b = i//2, heads h0 = (i%2)*8 .. +8
(data parallel on B, tensor parallel over heads).  Each core's work is fully
local; the host sums the two partial projections per batch.

Dataflow is fully "transposed" so no on-chip transposes are ever needed:
  host feeds xT = hidden[b].T                       [E, S]
  Q^T,K^T = (Wq|Wk block)^T @ xT     -> [d, tok] per head   (W stationary)
  V       = xT_block^T @ Wv          -> [tok, d] natural    (xT stationary)
  S^T     = K^T_blk^T @ Q^T          -> [k, q]   (softmax dim on partitions)
  P^T     = exp(0.125*S^T + mask[k]) * causal01
  sums    = ones^T @ P^T             -> [1, q]  (ones col in V, PSUM-accum)
  attn^T  = V_blk^T @ P^T            -> [d, q]  accumulated over k tiles
  norm    = attn^T * broadcast(1/sums)   (broadcast via K=1 ones-matmul)
  out^T   = Wp_blk^T @ attn^T        -> [col, tok]
Host transposes out^T back and sums core pairs + b_proj.

Schedule: the PE (matmul) engine is the global bottleneck (~200us of work
vs ~160us ACT exp).  Attention k-loops are exp-paced, so PE idle slots are
filled from a two-tier filler queue: prep (next quarter's V/Q/K) drains
first; deferred projections are RESERVED for the last quarter, whose
attention stretch has the largest ACT-vs-PE deficit.  Input DMAs are split
across the two hardware DGE rings (SP + Activation) and ordered so the
first matmul group gates on ~2MB, not the full 9MB.
"""

import os
import ml_dtypes
import numpy as np

B, S, E, H, D = 4, 2048, 1024, 16, 64
NC = 8
HL = H // 2          # local heads per core
EL = HL * D          # local embedding slice = 512
P = 128              # partitions
QT = 512             # q tile width (f32 moving max)
NQT = S // QT        # 4 q tiles
NKT = S // P         # 16 k tiles
NET = E // P         # 8 e (contraction) tiles

_CACHE = {}
LAST_RESULT = None


def _build(has_bv: bool):
    from contextlib import ExitStack

    import concourse.tile as tile
    from concourse import bacc, mybir

    f32 = mybir.dt.float32
    f32r = mybir.dt.bfloat16  # matmul operand dtype (2-byte: full-rate moving operand)
    EXP = mybir.ActivationFunctionType.Exp

    nc = bacc.Bacc(
        "TRN2",
        target_bir_lowering=False,
        debug=False,
        enable_asserts=False,
        num_devices=NC,
    )

    def inp(name, shape, dt=f32):
        return nc.dram_tensor(name, shape, dt, kind="ExternalInput").ap()

    xt_d = inp("xt", [E, S], f32r)
    wq_d = inp("wq", [E, EL], f32r)
    wk_d = inp("wk", [E, EL], f32r)
    wv_d = inp("wv", [E, EL], f32r)
    wp_d = inp("wp", [EL, E], f32r)
    constf_d = inp("constf", [P, 28])          # bq | bk | bv | maskt
    constr_d = inp("constr", [P, 192], f32r)   # causal | ones
    out_d = nc.dram_tensor("out", [E, S], f32r, kind="ExternalOutput").ap()

    with tile.TileContext(nc) as tc, ExitStack() as ctx:
        const = ctx.enter_context(tc.tile_pool(name="const", bufs=1))
        big = ctx.enter_context(tc.tile_pool(name="big", bufs=1))
        wpool = ctx.enter_context(tc.tile_pool(name="wpool", bufs=1))
        xpool = ctx.enter_context(tc.tile_pool(name="xpool", bufs=1))
        ptpool = ctx.enter_context(tc.tile_pool(name="ptpool", bufs=1))
        aopool = ctx.enter_context(tc.tile_pool(name="aopool", bufs=1))
        ospool = ctx.enter_context(tc.tile_pool(name="ospool", bufs=1))
        rcpool = ctx.enter_context(tc.tile_pool(name="rcpool", bufs=1))
        aospool = ctx.enter_context(tc.tile_pool(name="aospool", bufs=1))
        psum = ctx.enter_context(tc.tile_pool(name="psum", bufs=1, space="PSUM"))

        # ---- persistent big buffers ----
        # Q^T / K^T: per head-pair p a [128, S] tile (partitions = 2 heads x 64 d)
        qt_tiles = [big.tile([P, S], f32r, name=f"qt{p}", tag=f"qt{p}") for p in range(4)]
        kt_tiles = [big.tile([P, S], f32r, name=f"kt{p}", tag=f"kt{p}") for p in range(4)]
        # V natural: 16 tiles [128 tok, 512 vcol (+ones col per head)]
        v_tiles = [big.tile([P, 8 * 65], f32r, name=f"v{t}", tag=f"v{t}") for t in range(NKT)]

        x_tiles = [[None] * NQT for _ in range(NET)]

        # ---- DMA emission on two hardware DGE rings, ordered by first use.
        # Activation ring: consts + Wq/Wk (gate the first Q/K groups).
        constf_t = const.tile([P, 28], f32, name="constf_t")
        nc.scalar.dma_start(constf_t[:], constf_d[:])
        constr_t = const.tile([P, 192], f32r, name="constr_t")
        nc.scalar.dma_start(constr_t[:], constr_d[:])
        bq_t = constf_t[:, 0:4]
        bk_t = constf_t[:, 4:8]
        bv_t = constf_t[:, 8:12]
        maskt_t = constf_t[:, 12:28]
        causal_t = constr_t[:, 0:128]
        ones_t = constr_t[:, 128:192]

        def load_w_big(engine, dram, label):
            wb = wpool.tile([P, NET * EL], f32r, name=f"wb_{label}",
                            tag=f"wb_{label}", bufs=1)
            engine.dma_start(
                wb[:].rearrange("p (a c) -> p a c", a=NET, c=EL),
                dram.rearrange("(a p) c -> p a c", p=P),
            )
            return [wb[:, kt * EL:(kt + 1) * EL] for kt in range(NET)]

        def load_x_quarter(tq):
            xb = xpool.tile([P, NET * QT], f32r, name=f"xb{tq}", tag=f"xb{tq}", bufs=1)
            nc.sync.dma_start(
                xb[:].rearrange("p (a c) -> p a c", a=NET, c=QT),
                xt_d.rearrange("(a p) s -> p a s", p=P)[:, :, tq * QT:(tq + 1) * QT],
            )
            for kt in range(NET):
                x_tiles[kt][tq] = xb[:, kt * QT:(kt + 1) * QT]

        wq_t = load_w_big(nc.scalar, wq_d, "q")
        wk_t = load_w_big(nc.scalar, wk_d, "k")
        # SP ring: x quarters (feed everything downstream), wv, then wp.
        load_x_quarter(0)
        wv_t = load_w_big(nc.sync, wv_d, "v")
        load_x_quarter(1)
        load_x_quarter(2)
        load_x_quarter(3)
        wpb = wpool.tile([P, 4 * E], f32r, name="wpb", tag="wpb", bufs=1)
        nc.sync.dma_start(
            wpb[:].rearrange("p (a c) -> p a c", a=4, c=E),
            wp_d.rearrange("(a p) c -> p a c", p=P),
        )
        wp_tiles = [wpb[:, p * E:(p + 1) * E] for p in range(4)]

        # ---- per-group compute units (run directly or as fillers) ----
        done = set()

        def v_tt(tq, tt):
            key = ("v", tq, tt)
            if key in done:
                return
            done.add(key)
            ps = psum.tile([P, EL], f32, name=f"psv{tq}_{tt}", tag="mm", bufs=2)
            for kt in range(NET):
                nc.tensor.matmul(
                    ps[:], x_tiles[kt][tq][:, tt * P:(tt + 1) * P], wv_t[kt][:],
                    start=(kt == 0), stop=(kt == NET - 1))
            vt = v_tiles[tq * 4 + tt]
            v8 = vt[:, 0:520].rearrange("p (a c) -> p a c", a=8, c=65)
            nc.vector.tensor_copy(
                v8[:, :, 0:64], ps[:].rearrange("p (a c) -> p a c", a=8, c=64))
            nc.gpsimd.memset(v8[:, :, 64:65], 1.0)

        def q_ct(tq, ct):
            key = ("q", tq, ct)
            if key in done:
                return
            done.add(key)
            ps = psum.tile([P, QT], f32, name=f"psq{tq}_{ct}", tag="mm", bufs=2)
            for kt in range(NET):
                nc.tensor.matmul(ps[:], wq_t[kt][:, ct * P:(ct + 1) * P],
                                 x_tiles[kt][tq][:],
                                 start=(kt == 0), stop=(kt == NET - 1))
            nc.vector.tensor_scalar_add(
                qt_tiles[ct][:, tq * QT:(tq + 1) * QT], ps[:], bq_t[:, ct:ct + 1])

        def k_ct(tq, ct):
            key = ("k", tq, ct)
            if key in done:
                return
            done.add(key)
            ps = psum.tile([P, QT], f32, name=f"psk{tq}_{ct}", tag="mm", bufs=2)
            for kt in range(NET):
                nc.tensor.matmul(ps[:], wk_t[kt][:, ct * P:(ct + 1) * P],
                                 x_tiles[kt][tq][:],
                                 start=(kt == 0), stop=(kt == NET - 1))
            nc.vector.tensor_scalar_add(
                kt_tiles[ct][:, tq * QT:(tq + 1) * QT], ps[:], bk_t[:, ct:ct + 1])

        ao_map = {}

        def proj_ct(qt, ct):
            key = ("p", qt, ct)
            if key in done:
                return
            done.add(key)
            ps = psum.tile([P, QT], f32, name=f"psp{qt}_{ct}", tag="mm", bufs=2)
            for p in range(4):
                nc.tensor.matmul(ps[:], wp_tiles[p][:, ct * P:(ct + 1) * P],
                                 ao_map[(p, qt)][:], start=(p == 0), stop=(p == 3))
            osb = ospool.tile([P, QT], f32r, name=f"os{qt}_{ct}", tag="os", bufs=2)
            nc.vector.tensor_copy(osb[:], ps[:])
            nc.sync.dma_start(out_d[ct * P:(ct + 1) * P, qt * QT:(qt + 1) * QT],
                              osb[:])

        # ---- two-tier filler queue ----
        prep_fillers = []   # V/Q/K groups: drain first, anywhere
        late_fillers = []   # deferred proj groups: reserved for last quarter

        def mkfiller(fn, *args):
            def run():
                before = len(done)
                fn(*args)
                return len(done) != before
            return run

        def drain_one(allow_late):
            while prep_fillers:
                fn = prep_fillers.pop(0)
                if fn():
                    return
            if allow_late:
                while late_fillers:
                    fn = late_fillers.pop(0)
                    if fn():
                        return

        def attention(p, qt, sga, sgb, allow_late, drain_mod):
            """Head pair p (heads 2p, 2p+1), q tile qt.

            Leaves attnout halves in an SBUF tile (bf16) and the softmax
            denominators in rows 32*p of sga/sgb.  Normalization happens
            batched per qt in normalize()."""
            kt_max = 4 * (qt + 1)
            # row 64 of each av accumulates the softmax denominator (ones col)
            ava = psum.tile([65, QT], f32, name=f"ava{p}_{qt}", tag="ava", bufs=1)
            avb = psum.tile([65, QT], f32, name=f"avb{p}_{qt}", tag="avb", bufs=1)

            def av_sums(kt, pt, off):
                first, last = kt == 0, kt == kt_max - 1
                vva = v_tiles[kt][:, (2 * p) * 65:(2 * p + 1) * 65]
                vvb = v_tiles[kt][:, (2 * p + 1) * 65:(2 * p + 2) * 65]
                nc.tensor.matmul(ava[:, off:QT], vva, pt[:, off:QT],
                                 start=first, stop=last)
                nc.tensor.matmul(avb[:, off:QT], vvb, pt[:, QT + off:2 * QT],
                                 start=first, stop=last)

            pending = None
            for kt in range(kt_max):
                # diagonal tiles: only q columns >= off are unmasked
                diag = kt >= qt * 4
                off = (kt - qt * 4) * P if diag else 0
                kl = slice(kt * P, (kt + 1) * P)
                qv = slice(qt * QT + off, (qt + 1) * QT)
                st = psum.tile([P, 2 * QT], f32, name=f"st{p}_{qt}_{kt}",
                               tag="st", bufs=2)
                nc.tensor.matmul(st[:, off:QT], kt_tiles[p][0:64, kl],
                                 qt_tiles[p][0:64, qv])
                nc.tensor.matmul(st[:, QT + off:2 * QT], kt_tiles[p][64:128, kl],
                                 qt_tiles[p][64:128, qv])
                pt = ptpool.tile([P, 2 * QT], f32r, name=f"pt{p}_{qt}_{kt}",
                                 tag="pt", bufs=5)
                bias = maskt_t[:, kt:kt + 1]
                if not diag or off == 0:
                    nc.scalar.activation(pt[:], st[:], EXP, bias=bias, scale=0.125)
                else:
                    stv = st[:].rearrange("p (h q) -> p h q", h=2, q=QT)[:, :, off:QT]
                    ptv = pt[:].rearrange("p (h q) -> p h q", h=2, q=QT)[:, :, off:QT]
                    nc.scalar.activation(ptv, stv, EXP, bias=bias, scale=0.125)
                if diag:
                    # triangular band at the leading 128 valid columns
                    nc.vector.tensor_mul(pt[:, off:off + P], pt[:, off:off + P],
                                         causal_t[:])
                    nc.vector.tensor_mul(pt[:, QT + off:QT + off + P],
                                         pt[:, QT + off:QT + off + P], causal_t[:])
                if pending is not None:
                    av_sums(*pending)
                    if kt % drain_mod == 0:
                        drain_one(allow_late)
                pending = (kt, pt, off)
            av_sums(*pending)

            # drain PSUM immediately so the next pair's AV can start
            aos = aospool.tile([P, QT], f32r, name=f"aos{p}_{qt}",
                               tag=f"aos{p}", bufs=2)
            nc.vector.tensor_copy(aos[0:64, :], ava[0:64, :])
            nc.vector.tensor_copy(aos[64:128, :], avb[0:64, :])
            row = 32 * p
            nc.vector.tensor_copy(sga[row:row + 1, :], ava[64:65, :])
            nc.vector.tensor_copy(sgb[row:row + 1, :], avb[64:65, :])
            return aos

        def normalize(qt, sga, sgb, aos_tiles):
            """Batched softmax normalization for all 4 pairs of one q tile."""
            rcf = rcpool.tile([97, QT], f32, name=f"rcf{qt}", tag="rcf", bufs=1)
            rcg = rcpool.tile([97, QT], f32, name=f"rcg{qt}", tag="rcg", bufs=1)
            nc.vector.reciprocal_approx_fast(rcf[:], sga[:])
            nc.vector.reciprocal_approx_fast(rcg[:], sgb[:])
            rca = rcpool.tile([97, QT], f32r, name=f"rca{qt}", tag="rca", bufs=1)
            rcb = rcpool.tile([97, QT], f32r, name=f"rcb{qt}", tag="rcb", bufs=1)
            nc.vector.tensor_copy(rca[:], rcf[:])
            nc.vector.tensor_copy(rcb[:], rcg[:])
            for p in range(4):
                row = 32 * p
                ao = aopool.tile([P, QT], f32r, name=f"ao{p}_{qt}",
                                 tag=f"ao{p}", bufs=4)
                for half, rcx in ((0, rca), (1, rcb)):
                    rb = psum.tile([64, QT], f32, name=f"rb{p}_{qt}_{half}",
                                   tag="mm", bufs=2)
                    nc.tensor.matmul(rb[:], ones_t[row:row + 1, 0:64],
                                     rcx[row:row + 1, :], tile_position=(row, 0))
                    nc.vector.tensor_mul(ao[64 * half:64 * (half + 1), :], rb[:],
                                         aos_tiles[p][64 * half:64 * (half + 1), :])
                if has_bv:
                    nc.vector.tensor_scalar_add(ao[:], ao[:], bv_t[:, p:p + 1])
                ao_map[(p, qt)] = ao

        # ============ main schedule ============
        pending_np = None
        for tq in range(NQT):
            # mandatory prelude: first pair's Q/K (gates the first scores),
            # then V; later pairs become fillers drained just in time
            q_ct(tq, 0)
            k_ct(tq, 0)
            for tt in range(4):
                v_tt(tq, tt)
            for ct in range(1, 4):
                prep_fillers.append(mkfiller(q_ct, tq, ct))
                prep_fillers.append(mkfiller(k_ct, tq, ct))
            if tq + 1 < NQT:
                for tt in range(4):
                    prep_fillers.append(mkfiller(v_tt, tq + 1, tt))
                for ct in range(4):
                    prep_fillers.append(mkfiller(q_ct, tq + 1, ct))
                    prep_fillers.append(mkfiller(k_ct, tq + 1, ct))
            allow_late = tq == NQT - 1
            drain_mod = 2 if tq == 0 else 1  # startup DMAs still landing in tq0
            sga = rcpool.tile([97, QT], f32, name=f"sga{tq}", tag="sga", bufs=2)
            sgb = rcpool.tile([97, QT], f32, name=f"sgb{tq}", tag="sgb", bufs=2)
            aos_tiles = []
            for p in range(4):
                q_ct(tq, p)
                k_ct(tq, p)
                aos_tiles.append(attention(p, tq, sga, sgb, allow_late, drain_mod))
                if p == 1 and pending_np is not None:
                    qt_prev = pending_np[0]
                    normalize(*pending_np)
                    for ct in range(NET):
                        late_fillers.append(mkfiller(proj_ct, qt_prev, ct))
                    pending_np = None
            pending_np = (tq, sga, sgb, aos_tiles)

        # final: leftover fillers, then last quarter's normalize + proj
        for fn in list(prep_fillers):
            fn()
        prep_fillers.clear()
        for fn in list(late_fillers):
            fn()
        late_fillers.clear()
        normalize(*pending_np)
        for ct in range(NET):
            proj_ct(NQT - 1, ct)

    nc.compile()
    return nc


def _causal_tiles():
    """[128, 128] lower-triangular 0/1 band mask (dq >= dk)."""
    dk = np.arange(P)[:, None]
    dq = np.arange(P)[None, :]
    return np.ascontiguousarray((dq >= dk).astype(np.float32))


def kernel(hidden_state, attention_mask, W_attn, b_attn, W_proj, b_proj):
    global LAST_RESULT
    hs = np.asarray(hidden_state, np.float32)
    am = np.asarray(attention_mask, np.float32).reshape(B, S)
    wa = np.asarray(W_attn, np.float32)
    ba = np.asarray(b_attn, np.float32)
    wpr = np.asarray(W_proj, np.float32)
    bp = np.asarray(b_proj, np.float32)

    has_bv = bool(np.any(ba[2 * E:3 * E] != 0.0))
    key = ("k", has_bv)
    if key not in _CACHE:
        _CACHE[key] = _build(has_bv)
    nc = _CACHE[key]

    bf16 = ml_dtypes.bfloat16
    causal = _causal_tiles().astype(bf16)
    constr = np.concatenate([causal, np.ones((P, 64), bf16)], axis=1)
    constr = np.ascontiguousarray(constr)
    in_maps = []
    for core in range(NC):
        b = core // 2
        c0 = (core % 2) * EL
        constf = np.concatenate(
            [
                ba[c0:c0 + EL].reshape(4, P).T,
                ba[E + c0:E + c0 + EL].reshape(4, P).T,
                ba[2 * E + c0:2 * E + c0 + EL].reshape(4, P).T,
                am[b].reshape(NKT, P).T,
            ],
            axis=1,
        ).astype(np.float32)
        in_maps.append({
            "xt": np.ascontiguousarray(hs[b].T).astype(bf16),
            "wq": np.ascontiguousarray(wa[:, c0:c0 + EL]).astype(bf16),
            "wk": np.ascontiguousarray(wa[:, E + c0:E + c0 + EL]).astype(bf16),
            "wv": np.ascontiguousarray(wa[:, 2 * E + c0:2 * E + c0 + EL]).astype(bf16),
            "wp": np.ascontiguousarray(wpr[c0:c0 + EL, :]).astype(bf16),
            "constf": np.ascontiguousarray(constf),
            "constr": constr,
        })

    from concourse.bass_utils import run_bass_kernel_spmd

    trace = os.environ.get("KERNEL_TRACE", "") == "1"
    res = run_bass_kernel_spmd(nc, in_maps, core_ids=list(range(NC)), trace=trace)
    LAST_RESULT = res

    full = np.empty((B, S, E), np.float32)
    for b in range(B):
        full[b] = res.results[2 * b]["out"].T.astype(np.float32)
        full[b] += res.results[2 * b + 1]["out"].T.astype(np.float32)
        full[b] += bp
    return full
